# revision 28
# baseline (speedup 1.0000x reference)
"""GAT message-passing model on 8 Trainium2 NeuronCores.

Strategy (v3): edges sorted by destination on the host; nodes split into 8
contiguous ranges balanced by incoming-edge count (one per core).  Each HBM
core-pair builds ONE shared k/v node table (addr_space="Shared"): the even
core writes table rows [0,25088), the odd core [25088,50176), via direct
DMAs whose row offset comes from a partition_id()-derived register, then a
tiny pairwise AllGather acts as the cross-core barrier.  Per-window q
projections stay resident in SBUF.  The edge phase processes windows of
<=128 dst nodes / <=1024 edges: one fused input-stream DMA per window
(one-hots both orientations, transposed edge features, gather indices),
one indirect gather per window for all 8 edge-tiles' k|v rows; per tile
the kernel computes kkeT = We^T@efT + k^T (PE, identity-matmul transpose
trick), qeT via host-precomputed one-hots (PE), prodT = qeT*kkeT (DVE,
straight from both PSUM banks), per-head logits via head-selector
reduction matmuls (PE), logits un-transposed by a tiny matmul, exp+head-
expansion fused in one ACT op, w*v on DVE (2x mode), and segment-sums into
PSUM via one-hot matmuls.  Window finalize normalizes agg/den and dots
with Wd into a per-window column of an SBUF accumulator; one sigmoid + one
DMA at the very end write all window slots, which the host scatters back
to node order.
"""

import numpy as np
import ml_dtypes

import concourse.bass as bass
import concourse.bacc as bacc
import concourse.mybir as mybir
import concourse.tile as tile

BF16 = ml_dtypes.bfloat16

N_NODES = 50000
H, DH = 8, 64
DOUT = H * DH  # 512
N_CORES = 8
TPW = 8           # edge tiles per window
PAIR_SHARED = True
HLEN = 25088      # 196 chunks of 128; rows 50000..50175 are zero pads
TRASH = 50000     # zero row of the table (odd half's pad region)

# packed constant layout (columns of the single wconst input)
C_WQ = 0
C_WK = C_WQ + 2 * DOUT
C_WV = C_WK + 2 * DOUT
C_WD = C_WV + 2 * DOUT
C_ID = C_WD + DOUT
C_WE = C_ID + 128
C_ID8 = C_WE + DOUT
C_HS = C_ID8 + 8
C_END = C_HS + 4 * 8

# fused per-window stream layout (bf16 columns)
S_OHT = 0                  # [128, 1024]  one-hot [n, e] per tile block
S_OHE = 1024               # [128, 1024]  one-hot [e, n] per tile block
S_EFT = 2048               # [64, 1024]   edge features transposed
S_GS = 3072                # [128, 16]    gather indices (int32 as 2xbf16)
S_END = 3088


# ----------------------------------------------------------------------------
# Host-side planning
# ----------------------------------------------------------------------------

def make_plan(src, dst, n_nodes, n_cores, tpw):
    E = src.shape[0]
    perm = np.argsort(dst, kind="stable")
    s_src = src[perm]
    s_dst = dst[perm]
    deg = np.bincount(dst, minlength=n_nodes)
    cum = np.concatenate([[0], np.cumsum(deg)])

    cuts = [0]
    for c in range(1, n_cores):
        target = c * E / n_cores
        n = int(np.searchsorted(cum, target))
        n = max(cuts[-1] + 1, min(n, n_nodes - (n_cores - c)))
        cuts.append(n)
    cuts.append(n_nodes)

    cores = []
    for c in range(n_cores):
        nlo, nhi = cuts[c], cuts[c + 1]
        wins = []
        n = nlo
        while n < nhi:
            n2 = n
            edges = 0
            while n2 < nhi and (n2 - n) < 128:
                if edges + deg[n2] > tpw * 128:
                    break
                edges += deg[n2]
                n2 += 1
            assert n2 > n, f"node {n} degree {deg[n]} > {tpw*128}"
            wins.append((n, n2))
            n = n2
        cores.append(dict(nlo=nlo, nhi=nhi, wins=wins))

    NWIN = max(len(c["wins"]) for c in cores)
    return dict(cores=cores, NWIN=NWIN, s_src=s_src, s_dst=s_dst,
                perm=perm, cum=cum)


def make_core_inputs(plan, core_idx, ef_sorted, nfT):
    """Per-core edge-phase inputs: fused stream + per-window q features."""
    NWIN = plan["NWIN"]
    core = plan["cores"][core_idx]
    cum = plan["cum"]
    DE = ef_sorted.shape[1]
    NW4 = (NWIN + 3) // 4

    strm = np.zeros((NWIN, 128, S_END), BF16)
    gsrc = np.full((NWIN, 128, TPW), TRASH, np.int32)
    nfT_w = np.zeros((NW4, 128, 4 * 256), BF16)

    for w, (a, b) in enumerate(core["wins"]):
        e0, e1 = cum[a], cum[b]
        cnt = e1 - e0
        sl = np.arange(cnt)
        t_idx = sl // 128
        p_idx = sl % 128
        gsrc[w, p_idx, t_idx] = plan["s_src"][e0:e1]
        dl = (plan["s_dst"][e0:e1] - a).astype(np.int64)
        strm[w, dl, S_OHT + t_idx * 128 + p_idx] = 1
        strm[w, p_idx, S_OHE + t_idx * 128 + dl] = 1
        strm[w, :DE, S_EFT + t_idx * 128 + p_idx] = \
            ef_sorted[e0:e1].astype(BF16)
        L = b - a
        nfT_w[w // 4, :, (w % 4) * 256:(w % 4) * 256 + L] = nfT[:128, a:b]
        nfT_w[w // 4, :, (w % 4) * 256 + 128:(w % 4) * 256 + 128 + L] = \
            nfT[128:256, a:b]
    strm[:, :, S_GS:S_END] = gsrc.view(np.uint16).view(BF16).reshape(
        NWIN, 128, 16)
    return dict(strm=strm, nfT_w=nfT_w, wins=core["wins"])


def make_table_inputs(core_idx, nfT, n_nodes, pair_shared):
    """Table-build node-feature chunks (x4-fused): this core's rows."""
    if pair_shared:
        half = core_idx % 2
        r0 = half * HLEN
        r1 = min(r0 + HLEN, n_nodes)
        hchunk = HLEN // 128
    else:
        r0, r1 = 0, n_nodes
        hchunk = 2 * HLEN // 128
    assert hchunk % 4 == 0
    nfT_h = np.zeros((hchunk // 4, 128, 4 * 256), BF16)
    for i in range(hchunk):
        a = r0 + i * 128
        b = min(a + 128, r1)
        if b > a:
            L = b - a
            c0 = (i % 4) * 256
            nfT_h[i // 4, :, c0:c0 + L] = nfT[:128, a:b]
            nfT_h[i // 4, :, c0 + 128:c0 + 128 + L] = nfT[128:256, a:b]
    return dict(nfT_h=nfT_h, HCHUNK=hchunk)


def make_global_inputs(nf, Wq, Wk, Wv, We, Wd):
    N, DIN = nf.shape
    nfT = nf.T.astype(BF16)
    scale = 1.0 / np.sqrt(DH)

    def pack_w(W):
        return np.concatenate([W[:128], W[128:256]], axis=1).astype(BF16)

    wconst = np.zeros((128, C_END), BF16)
    wconst[:, C_WQ:C_WQ + 2 * DOUT] = pack_w(Wq * scale)
    wconst[:, C_WK:C_WK + 2 * DOUT] = pack_w(Wk)
    wconst[:, C_WV:C_WV + 2 * DOUT] = pack_w(Wv)
    wconst[:, C_WD:C_WD + DOUT] = np.tile(Wd.reshape(1, DOUT), (128, 1))
    wconst[:, C_ID:C_ID + 128] = np.eye(128)
    wconst[:64, C_WE:C_WE + DOUT] = We.astype(BF16)
    wconst[:8, C_ID8:C_ID8 + 8] = np.eye(8)
    # hsel[c][d, h] = 1 iff head h == 2c + (d >= 64): reduction selectors
    # that drop chunk c's two heads into rows 2c / 2c+1 of the logits PSUM.
    for c in range(4):
        wconst[:64, C_HS + c * 8 + 2 * c] = 1
        wconst[64:, C_HS + c * 8 + 2 * c + 1] = 1
    return dict(wconst=wconst, nfT=nfT, N=N)


# ----------------------------------------------------------------------------
# Device kernel emission (identical instruction stream on every core)
# ----------------------------------------------------------------------------

def build_nc(NWIN, HCHUNK, bd0, pair_shared=PAIR_SHARED):
    import os
    STAGE = int(os.environ.get("KSTAGE", "3"))  # 1=table,2=+q,3=full
    dt = mybir.dt
    bf16, f32, i32 = dt.bfloat16, dt.float32, dt.int32
    NROW = 2 * HLEN + 128
    NW4 = (NWIN + 3) // 4
    HB = HCHUNK // 4

    nc = bacc.Bacc("TRN2", target_bir_lowering=False, debug=False)

    t_wc = nc.dram_tensor("wconst", [128, C_END], bf16, kind="ExternalInput")
    t_nfT_h = nc.dram_tensor("nfT_h", [HB, 128, 4 * 256], bf16, kind="ExternalInput")
    t_nfT_w = nc.dram_tensor("nfT_w", [NW4, 128, 4 * 256], bf16, kind="ExternalInput")
    t_strm = nc.dram_tensor("strm", [NWIN, 128, S_END], bf16, kind="ExternalInput")

    t_y = nc.dram_tensor("y_out", [128, NWIN], f32, kind="ExternalOutput")
    DBG = int(os.environ.get("KDBG", "0"))
    t_dbg = {}
    if DBG:
        for nm, shp, dtp in [("d_kvw", [128, 2048], bf16),
                             ("d_kke", [128, 1024], bf16),
                             ("d_prod", [128, 1024], bf16),
                             ("d_wT", [8, 256], bf16),
                             ("d_wsb", [128, 16], bf16),
                             ("d_wv", [128, 512], bf16),
                             ("d_q", [128, 512], bf16),
                             ("d_aggsb", [128, 512], bf16),
                             ("d_xn", [128, 512], bf16)]:
            t_dbg[nm] = nc.dram_tensor(nm, shp, dtp, kind="ExternalOutput")

    t_kv = nc.dram_tensor("kv_table", [NROW, 2 * DOUT], bf16, kind="Internal",
                          addr_space="Shared" if pair_shared else "Local")
    if pair_shared:
        t_cc_in = nc.dram_tensor("cc_in", [1, 4], i32, kind="Internal")
        t_cc_out = nc.dram_tensor("cc_out", [2, 4], i32, kind="Internal")

    with tile.TileContext(nc, pool_alloc_mode="queue") as tc:
        with tc.tile_pool(name="wpool", bufs=1) as wpool:
            wc = wpool.tile([128, C_END], bf16)
            nc.sync.dma_start(out=wc[:], in_=t_wc[:])
            wq_sb = wc[:, C_WQ:C_WQ + 2 * DOUT]
            wk_sb = wc[:, C_WK:C_WK + 2 * DOUT]
            wv_sb = wc[:, C_WV:C_WV + 2 * DOUT]
            wdrow_sb = wc[:, C_WD:C_WD + DOUT]
            ident_sb = wc[:, C_ID:C_ID + 128]
            we_sb = wc[:64, C_WE:C_WE + DOUT]
            ident8_sb = wc[:8, C_ID8:C_ID8 + 8]
            hsel_sb = [wc[:, C_HS + c * 8:C_HS + (c + 1) * 8] for c in range(4)]
            q_all = wpool.tile([128, NWIN * DOUT], bf16)
            y_acc = wpool.tile([128, NWIN], f32)

            # slot base register: pair half = partition_id() % 2
            if pair_shared:
                pid = nc.sync.partition_id()
                base = (pid % 2) * HB
            else:
                base = 0

            # ---------------- phase 1: k/v table (this core's share) --------
            table_writes = []
            with tc.tile_pool(name="p1", bufs=4) as p1, \
                 tc.tile_pool(name="p1ps", bufs=2, space="PSUM") as p1ps:
                for i4 in range(HB):
                    xt = p1.tile([128, 4 * 256], bf16, tag="xt")
                    nc.scalar.dma_start(out=xt[:], in_=t_nfT_h[i4])
                    kv4 = p1.tile([128, 4 * 2 * DOUT], bf16, tag="kv4")
                    for s in range(4):
                        ps_kv = p1ps.tile([128, 2 * DOUT], f32, tag="kv", bufs=3)
                        for c in range(2):
                            nc.tensor.matmul(
                                ps_kv[:, :DOUT],
                                xt[:, s * 256 + c * 128:s * 256 + (c + 1) * 128],
                                wk_sb[:, c * DOUT:(c + 1) * DOUT],
                                start=(c == 0), stop=(c == 1))
                            nc.tensor.matmul(
                                ps_kv[:, DOUT:],
                                xt[:, s * 256 + c * 128:s * 256 + (c + 1) * 128],
                                wv_sb[:, c * DOUT:(c + 1) * DOUT],
                                start=(c == 0), stop=(c == 1))
                        o = s * 2 * DOUT
                        nc.vector.tensor_copy(kv4[:, o:o + DOUT],
                                              ps_kv[:, :DOUT])
                        nc.scalar.copy(kv4[:, o + DOUT:o + 2 * DOUT],
                                       ps_kv[:, DOUT:])
                    dst = t_kv[bass.ts(base + i4, 512)].rearrange(
                        "(c p) e -> p c e", p=128)
                    table_writes.append(
                        nc.sync.dma_start(out=dst, in_=kv4[:].rearrange(
                            "p (c e) -> p c e", c=4)))

                # ---------------- phase 1b: per-window q (stays in SBUF) ----
                for w4 in range(NW4 if STAGE >= 2 else 0):
                    xt = p1.tile([128, 4 * 256], bf16, tag="xt")
                    nc.sync.dma_start(out=xt[:], in_=t_nfT_w[w4])
                    for s in range(4):
                        w = w4 * 4 + s
                        if w >= NWIN:
                            break
                        ps_q = p1ps.tile([128, DOUT], f32, tag="q", bufs=2)
                        for c in range(2):
                            nc.tensor.matmul(
                                ps_q[:],
                                xt[:, s * 256 + c * 128:s * 256 + (c + 1) * 128],
                                wq_sb[:, c * DOUT:(c + 1) * DOUT],
                                start=(c == 0), stop=(c == 1))
                        if w % 2 == 0:
                            nc.vector.tensor_copy(
                                q_all[:, w * DOUT:(w + 1) * DOUT], ps_q[:])
                        else:
                            nc.scalar.copy(
                                q_all[:, w * DOUT:(w + 1) * DOUT], ps_q[:])

            # Fence: all table writes must complete before any edge gather.
            fence_tile = wpool.tile([1, 4], i32)
            fence = nc.gpsimd.memset(fence_tile[:], 0)
            for wdma in table_writes:
                tile.add_dep_helper(fence.ins, wdma.ins, sync=True,
                                    reason="table fence")
            if pair_shared:
                # tiny pairwise AllGather = cross-core barrier for the pair
                cc_seed = wpool.tile([1, 4], i32)
                nc.gpsimd.memset(cc_seed[:], 0)
                seed_dma = nc.sync.dma_start(out=t_cc_in[:], in_=cc_seed[:])
                cc = nc.gpsimd.collective_compute(
                    kind="AllGather",
                    op=mybir.AluOpType.bypass,
                    replica_groups=[[0, 1], [2, 3], [4, 5], [6, 7]],
                    ins=[t_cc_in[:]],
                    outs=[t_cc_out[:]],
                )
                tile.add_dep_helper(cc.ins, fence.ins, sync=True,
                                    reason="barrier after table")
                tile.add_dep_helper(cc.ins, seed_dma.ins, sync=True,
                                    reason="barrier seed")
                gate = nc.gpsimd.memset(fence_tile[:], 1)
                tile.add_dep_helper(gate.ins, cc.ins, sync=True,
                                    reason="gate on barrier")
            else:
                gate = fence

            # ---------------- phase 2: edge phase ----------------
            with tc.tile_pool(name="p2", bufs=6) as p2, \
                 tc.tile_pool(name="p2s", bufs=3) as p2s, \
                 tc.tile_pool(name="p2w", bufs=2) as p2w, \
                 tc.tile_pool(name="psK", bufs=1, space="PSUM") as psK, \
                 tc.tile_pool(name="psQ", bufs=1, space="PSUM") as psQ, \
                 tc.tile_pool(name="psL", bufs=2, space="PSUM") as psL, \
                 tc.tile_pool(name="psA", bufs=1, space="PSUM") as psA:
                for w in range(NWIN if STAGE >= 3 else 0):
                    strm = p2s.tile([128, S_END], bf16, tag="strm")
                    nc.scalar.dma_start(out=strm[:], in_=t_strm[w])
                    kvw = p2w.tile([128, TPW * 2 * DOUT], bf16, tag="kvw")
                    gso = strm[:, S_GS:S_END].bitcast(i32)
                    for t in range(TPW):
                        g_ins = nc.gpsimd.indirect_dma_start(
                            out=kvw[:, t * 2 * DOUT:(t + 1) * 2 * DOUT],
                            out_offset=None, in_=t_kv[:],
                            in_offset=bass.IndirectOffsetOnAxis(
                                ap=gso[:, t:t + 1], axis=0))
                        tile.add_dep_helper(g_ins.ins, gate.ins, sync=True,
                                            reason="wait table barrier")
                    agg = psA.tile([128, DOUT], f32, tag="agg")
                    den = psA.tile([128, H], f32, tag="den")
                    for g in range(TPW // 2):
                        t0 = 2 * g
                        efT_2 = strm[:64, S_EFT + t0 * 128:
                                     S_EFT + (t0 + 2) * 128]
                        ohT_2 = strm[:, S_OHT + t0 * 128:S_OHT + (t0 + 2) * 128]
                        ps_k = psK.tile([128, 2 * DOUT], f32, tag="kke")
                        ps_q = psQ.tile([128, 2 * DOUT], f32, tag="qe")
                        for c in range(4):
                            co = c * 256
                            nc.tensor.matmul(
                                ps_k[:, co:co + 256],
                                we_sb[:, c * 128:(c + 1) * 128], efT_2,
                                start=True, stop=False)
                            for u in range(2):
                                t = t0 + u
                                nc.tensor.matmul(
                                    ps_k[:, co + u * 128:co + (u + 1) * 128],
                                    kvw[:, t * 2 * DOUT + c * 128:
                                        t * 2 * DOUT + (c + 1) * 128],
                                    ident_sb, start=False, stop=True)
                            nc.tensor.matmul(
                                ps_q[:, co:co + 256],
                                q_all[:, w * DOUT + c * 128:
                                      w * DOUT + (c + 1) * 128],
                                ohT_2, start=True, stop=True)
                        kke_sb = p2.tile([128, 2 * DOUT], bf16, tag="kke_sb")
                        nc.scalar.copy(kke_sb[:], ps_k[:])
                        if DBG and w == 0 and g == 0:
                            nc.gpsimd.dma_start(out=t_dbg["d_kvw"][:], in_=kvw[:, :2048])
                            nc.gpsimd.dma_start(out=t_dbg["d_kke"][:], in_=kke_sb[:])
                            nc.gpsimd.dma_start(out=t_dbg["d_q"][:], in_=q_all[:, :512])
                        prodT = p2.tile([128, 2 * DOUT], bf16, tag="prodT")
                        nc.vector.tensor_tensor(
                            prodT[:], ps_q[:], kke_sb[:], mybir.AluOpType.mult)
                        lw = psL.tile([128, 272], f32, tag="lw")
                        for c in range(4):
                            nc.tensor.matmul(
                                lw[:8, :256], hsel_sb[c],
                                prodT[:, c * 256:(c + 1) * 256],
                                start=(c == 0), stop=(c == 3))
                        wT_sb = p2.tile([8, 256], bf16, tag="wT")
                        nc.scalar.activation(wT_sb[:], lw[:8, :256],
                                             mybir.ActivationFunctionType.Exp)
                        for u in range(2):
                            nc.tensor.matmul(
                                lw[:, 256 + u * 8:256 + (u + 1) * 8],
                                wT_sb[:, u * 128:(u + 1) * 128], ident8_sb,
                                start=True, stop=True)
                        w_sb = p2.tile([128, 16], bf16, tag="w_sb")
                        nc.scalar.copy(w_sb[:], lw[:, 256:272])
                        if DBG and w == 0 and g == 0:
                            nc.gpsimd.dma_start(out=t_dbg["d_prod"][:], in_=prodT[:])
                            nc.gpsimd.dma_start(out=t_dbg["d_wT"][:], in_=wT_sb[:])
                            nc.gpsimd.dma_start(out=t_dbg["d_wsb"][:], in_=w_sb[:])
                        for u in range(2):
                            t = t0 + u
                            vslice = kvw[:, t * 2 * DOUT + DOUT:
                                         (t + 1) * 2 * DOUT]
                            ohe_t = strm[:, S_OHE + t * 128:
                                         S_OHE + (t + 1) * 128]
                            wv_t = p2.tile([128, DOUT], bf16, tag="wv")
                            if u == 0:
                                wx = p2.tile([128, H, DH], bf16, tag="wx")
                                nc.scalar.copy(
                                    wx[:], w_sb[:, t * 8 - t0 * 8:][:, :H]
                                    [:, :, None].to_broadcast([128, H, DH]))
                                nc.vector.tensor_tensor(
                                    wv_t[:], wx[:].rearrange("p h d -> p (h d)"),
                                    vslice, mybir.AluOpType.mult)
                            else:
                                nc.vector.tensor_tensor(
                                    wv_t[:].rearrange("p (h d) -> p h d", h=H),
                                    w_sb[:, u * 8:(u + 1) * 8][:, :, None]
                                    .to_broadcast([128, H, DH]),
                                    vslice.rearrange("p (h d) -> p h d", h=H),
                                    mybir.AluOpType.mult)
                            if DBG and w == 0 and t == 0:
                                nc.gpsimd.dma_start(out=t_dbg["d_wv"][:], in_=wv_t[:])
                            nc.tensor.matmul(
                                agg[:], ohe_t, wv_t[:],
                                start=(t == 0), stop=(t == TPW - 1))
                            nc.tensor.matmul(
                                den[:], ohe_t, w_sb[:, u * 8:(u + 1) * 8],
                                start=(t == 0), stop=(t == TPW - 1))
                    # window finalize: free agg/den ASAP, all-bf16 fast path
                    agg_sb = p2w.tile([128, DOUT], bf16, tag="agg_sb")
                    nc.scalar.copy(agg_sb[:], agg[:])
                    den_sb = p2w.tile([128, H], f32, tag="den_sb")
                    nc.vector.tensor_scalar_add(den_sb[:], den[:], 1e-9)
                    recip = p2w.tile([128, H], f32, tag="recip")
                    nc.vector.reciprocal(recip[:], den_sb[:])
                    rexp = p2w.tile([128, H, DH], bf16, tag="rexp")
                    nc.vector.tensor_copy(
                        rexp[:], recip[:, :, None].to_broadcast([128, H, DH]))
                    if DBG and w == 0:
                        nc.gpsimd.dma_start(out=t_dbg["d_aggsb"][:], in_=agg_sb[:])
                    xn = p2w.tile([128, DOUT], bf16, tag="xn")
                    nc.vector.tensor_tensor(
                        xn[:], agg_sb[:],
                        rexp[:].rearrange("p h d -> p (h d)"),
                        mybir.AluOpType.mult)
                    if DBG and w == 0:
                        nc.gpsimd.dma_start(out=t_dbg["d_xn"][:], in_=xn[:])
                    scr = p2w.tile([128, DOUT], bf16, tag="scr")
                    nc.vector.scalar_tensor_tensor(
                        out=scr[:], in0=xn[:], scalar=0.0, in1=wdrow_sb,
                        op0=mybir.AluOpType.max, op1=mybir.AluOpType.mult,
                        accum_out=y_acc[:, w:w + 1])
                # one sigmoid + one DMA for all windows
                if STAGE >= 3:
                    y_sb = wpool.tile([128, NWIN], f32)
                    nc.scalar.activation(y_sb[:], y_acc[:],
                                         mybir.ActivationFunctionType.Sigmoid,
                                         bias=float(bd0))
                    nc.sync.dma_start(out=t_y[:], in_=y_sb[:])
    nc.compile()
    return nc


# ----------------------------------------------------------------------------
# Entry point
# ----------------------------------------------------------------------------

LAST_RESULTS = None
LAST_NC = None


def kernel(node_features, edge_features, Wq, Wk, Wv, We, Wd, bd, src, dst,
           trace=False):
    from concourse.bass_utils import run_bass_kernel_spmd

    nf = np.asarray(node_features, dtype=np.float32)
    ef = np.asarray(edge_features, dtype=np.float32)
    src = np.asarray(src, dtype=np.int32)
    dst = np.asarray(dst, dtype=np.int32)
    Wq = np.asarray(Wq, np.float32)
    Wk = np.asarray(Wk, np.float32)
    Wv = np.asarray(Wv, np.float32)
    We = np.asarray(We, np.float32)
    Wd = np.asarray(Wd, np.float32)
    bd = np.asarray(bd, np.float32)
    N = nf.shape[0]

    plan = make_plan(src, dst, N, N_CORES, TPW)
    gin = make_global_inputs(nf, Wq, Wk, Wv, We, Wd)
    ef_sorted = ef[plan["perm"]]

    tin0 = make_table_inputs(0, gin["nfT"], N, PAIR_SHARED)
    HCHUNK = tin0["HCHUNK"]

    nc = build_nc(NWIN=plan["NWIN"], HCHUNK=HCHUNK,
                  bd0=float(bd.ravel()[0]), pair_shared=PAIR_SHARED)

    in_maps = []
    core_meta = []
    for c in range(N_CORES):
        cin = make_core_inputs(plan, c, ef_sorted, gin["nfT"])
        tin = make_table_inputs(c, gin["nfT"], N, PAIR_SHARED)
        m = dict(wconst=gin["wconst"], nfT_h=tin["nfT_h"],
                 strm=cin["strm"], nfT_w=cin["nfT_w"])
        in_maps.append(m)
        core_meta.append(cin["wins"])

    res = run_bass_kernel_spmd(nc, in_maps, core_ids=list(range(N_CORES)),
                               trace=trace)
    global LAST_RESULTS, LAST_NC
    LAST_RESULTS = res
    LAST_NC = nc

    y = np.zeros((N, 1), np.float32)
    for c, wins in enumerate(core_meta):
        yc = res.results[c]["y_out"]
        for w, (a, b) in enumerate(wins):
            y[a:b, 0] = yc[:b - a, w]
    return y
